# revision 1
# baseline (speedup 1.0000x reference)
"""GAT-style 2-layer knowledge-graph encoder on 8 trn2 NeuronCores.

Sharding: query rows, 512 per core. Scores are built transposed ([j, q]) so
the exp'd attention matrix is directly the matmul lhsT (no PE transposes).
The softmax denominator comes from a ones-column appended to the gathered
Wh payload (an extra matmul output column, no reduction pass). Wh for each
layer is computed on the owning shard and AllGathered on-device (bf16).
Masking is folded into two host-side edge-weight arrays:
  att = e * ew_pos + ew_neg   (ew_pos = ew on edges else 0,
                               ew_neg = -9e15*ew off edges else 0)
"""

import numpy as np
import ml_dtypes

import concourse.bass as bass
import concourse.bacc as bacc
import concourse.mybir as mybir
from concourse import tile, masks
from concourse.bass_utils import run_bass_kernel_spmd
from concourse.alu_op_type import AluOpType as alu

BF16 = mybir.dt.bfloat16
F32 = mybir.dt.float32

P = 128
NCORES = 8
N = 4096
NSH = 512          # rows per core
H = 4
DIN = 768
HID = 512
F1 = 2048
DOUT = 768
C0 = 514           # 512 Wh + ones + pad  (bf16)
C1 = 770           # 768 Wh + ones + pad  (bf16)
ALPHA = 0.2
NEGBIG = -9e15
EPS = 1e-5
NIB = NSH // P     # 4 row-blocks per core
CH = 4             # j-tiles per chunk
NCHUNK = (N // P) // CH
AF = mybir.ActivationFunctionType


def build_nc():
    nc = bacc.Bacc(num_devices=NCORES)

    nfT = nc.declare_dram_parameter("nfT", [DIN, NSH], F32, isOutput=False)
    ewpT = nc.declare_dram_parameter("ewpT", [N, NSH], F32, isOutput=False)
    ewnT = nc.declare_dram_parameter("ewnT", [N, NSH], BF16, isOutput=False)
    W0 = nc.declare_dram_parameter("W0", [H, DIN, HID], F32, isOutput=False)
    a0 = nc.declare_dram_parameter("a0", [1, H * 2 * HID], F32, isOutput=False)
    W1 = nc.declare_dram_parameter("W1", [H, F1, DOUT], F32, isOutput=False)
    a1 = nc.declare_dram_parameter("a1", [1, H * 2 * DOUT], F32,
                                   isOutput=False)
    rp0w = nc.declare_dram_parameter("rp0w", [DIN, F1], BF16, isOutput=False)
    rp0b = nc.declare_dram_parameter("rp0b", [1, F1], F32, isOutput=False)
    rp1w = nc.declare_dram_parameter("rp1w", [F1, DOUT], F32, isOutput=False)
    rp1b = nc.declare_dram_parameter("rp1b", [1, DOUT], F32, isOutput=False)
    ln0g = nc.declare_dram_parameter("ln0g", [1, F1], F32, isOutput=False)
    ln0b = nc.declare_dram_parameter("ln0b", [1, F1], F32, isOutput=False)
    ln1g = nc.declare_dram_parameter("ln1g", [1, DOUT], F32, isOutput=False)
    ln1b = nc.declare_dram_parameter("ln1b", [1, DOUT], F32, isOutput=False)
    h2 = nc.declare_dram_parameter("h2", [NSH, DOUT], F32, isOutput=True)

    g0_in = nc.dram_tensor("g0_in", [H, NSH, C0], BF16)
    g0_out = nc.dram_tensor("g0_out", [NCORES, H, NSH, C0], BF16)
    g0s_in = nc.dram_tensor("g0s_in", [H, NSH, 2], F32)
    g0s_out = nc.dram_tensor("g0s_out", [NCORES, H, NSH, 2], F32)
    g1_in = nc.dram_tensor("g1_in", [H, NSH, C1], BF16)
    g1_out = nc.dram_tensor("g1_out", [NCORES, H, NSH, C1], BF16)
    g1s_in = nc.dram_tensor("g1s_in", [H, NSH, 2], F32)
    g1s_out = nc.dram_tensor("g1s_out", [NCORES, H, NSH, 2], F32)

    groups = [list(range(NCORES))]

    with tile.TileContext(nc) as tc:
        with (
            tc.tile_pool(name="persist", bufs=1) as pp,
            tc.tile_pool(name="sb", bufs=2) as sb,
            tc.tile_pool(name="small", bufs=3) as sm,
        ):
            ident = pp.tile([P, P], F32)
            masks.make_identity(nc, ident[:])
            h2pre = pp.tile([P, NIB, DOUT], F32)

            def bcast(pool, dram_row, width, name):
                row = pool.tile([1, width], F32, tag="bc_row", bufs=1,
                                name=f"r_{name}")
                nc.sync.dma_start(out=row[:], in_=dram_row)
                out = pool.tile([P, width], F32, name=f"b_{name}")
                nc.gpsimd.partition_broadcast(out[:], row[0:1, :])
                return out

            def ln_elu(pool, x_ap, gb, bb, width, out_ap, do_elu):
                """LN over free dim; x_ap is clobbered as scratch (B0)."""
                b1 = pool.tile([P, width], F32, tag="ln_b1", bufs=1,
                               name="ln_b1")
                b2 = pool.tile([P, width], F32, tag="ln_b2", bufs=1,
                               name="ln_b2")
                s1 = sm.tile([P, 1], F32, tag="ln_s1", name="ln_s1")
                nc.vector.tensor_reduce(s1[:], x_ap, mybir.AxisListType.X,
                                        alu.add)
                negmean = sm.tile([P, 1], F32, tag="ln_nm", name="ln_nm")
                nc.vector.tensor_single_scalar(negmean[:], s1[:],
                                               -1.0 / width, alu.mult)
                nc.scalar.activation(b1[:], x_ap, AF.Identity,
                                     bias=negmean[:, 0:1])          # t
                ssq = sm.tile([P, 1], F32, tag="ln_ssq", name="ln_ssq")
                nc.scalar.activation(b2[:], b1[:], AF.Square,
                                     accum_out=ssq[:, 0:1])
                var = sm.tile([P, 1], F32, tag="ln_var", name="ln_var")
                nc.vector.tensor_scalar(var[:], ssq[:], 1.0 / width, EPS,
                                        alu.mult, alu.add)
                std = sm.tile([P, 1], F32, tag="ln_std", name="ln_std")
                nc.scalar.activation(std[:], var[:], AF.Sqrt)
                rstd = sm.tile([P, 1], F32, tag="ln_rstd", name="ln_rstd")
                nc.vector.reciprocal(rstd[:], std[:])
                nc.scalar.mul(b2[:], b1[:], rstd[:, 0:1])           # u
                nc.vector.tensor_tensor(b1[:], b2[:], gb, alu.mult)  # v
                if not do_elu:
                    nc.vector.tensor_tensor(out_ap, b1[:], bb, alu.add)
                    return
                nc.vector.tensor_tensor(b2[:], b1[:], bb, alu.add)   # w
                nc.vector.tensor_single_scalar(b1[:], b2[:], 0.0, alu.min)
                nc.scalar.activation(x_ap, b1[:], AF.Exp)            # -> B0
                nc.vector.tensor_single_scalar(b1[:], b2[:], 0.0, alu.max)
                nc.vector.scalar_tensor_tensor(out_ap, x_ap, -1.0, b1[:],
                                               alu.add, alu.add)

            def attention(lid, O, N1, g_out, gs_out, gs_in, dest, mean_heads):
                CX = O + 2
                with (
                    tc.tile_pool(name=f"att{lid}", bufs=1) as ap_,
                    tc.tile_pool(name=f"att{lid}_d", bufs=3) as ad,
                    tc.tile_pool(name=f"att{lid}_ps", bufs=1,
                                 space="PSUM") as aps,
                ):
                    ssb = []
                    for h in range(H):
                        row = sm.tile([1, NSH], F32, tag="ssrow",
                                      name=f"ssrow{lid}_{h}")
                        nc.sync.dma_start(
                            out=row[:],
                            in_=gs_in[h, :, 0:1].rearrange("q c -> c q"))
                        sbh = ap_.tile([P, NSH], F32, name=f"ssb{lid}_{h}")
                        nc.gpsimd.partition_broadcast(sbh[:], row[0:1, :])
                        ssb.append(sbh)
                    acc = [ap_.tile([P, NIB, O + 1], F32,
                                    name=f"acc{lid}_{hh}") for hh in range(H)]
                    whs = ap_.tile([P, CH, H, CX], BF16)
                    ewps = ap_.tile([P, CH, NSH], F32)
                    ewns = ap_.tile([P, CH, NSH], BF16)
                    svs = ap_.tile([P, CH, H, 2], F32)
                    for jc in range(NCHUNK):
                        for jt in range(CH):
                            jg = jc * CH + jt
                            s, r = jg // NIB, jg % NIB
                            nc.sync.dma_start(
                                out=whs[:, jt, :, :],
                                in_=g_out[s, :, r * P:(r + 1) * P, :]
                                .rearrange("h p c -> p h c"))
                            nc.sync.dma_start(
                                out=ewps[:, jt, :],
                                in_=ewpT[jg * P:(jg + 1) * P, :])
                            nc.sync.dma_start(
                                out=ewns[:, jt, :],
                                in_=ewnT[jg * P:(jg + 1) * P, :])
                            nc.sync.dma_start(
                                out=svs[:, jt, :, :],
                                in_=gs_out[s, :, r * P:(r + 1) * P, :]
                                .rearrange("h p c -> p h c"))
                        for h in range(H):
                            psa = [aps.tile([P, N1], F32, tag=f"psa{qb}",
                                            name=f"psa_{qb}")
                                   for qb in range(NIB)]
                            psb = [aps.tile([P, 257], F32, tag=f"psb{qb}",
                                            name=f"psb_{qb}")
                                   for qb in range(NIB)]
                            for jt in range(CH):
                                e = ad.tile([P, NSH], F32, tag="e", name="e")
                                nc.scalar.activation(
                                    e[:], ssb[h][:, :], AF.Lrelu,
                                    bias=svs[:, jt, h, 1:2], alpha=ALPHA)
                                att = ad.tile([P, NSH], F32, tag="att",
                                              name="att")
                                nc.vector.tensor_tensor(
                                    att[:], e[:], ewps[:, jt, :], alu.mult)
                                nc.vector.tensor_tensor(
                                    e[:], att[:], ewns[:, jt, :], alu.add)
                                pt = ad.tile([P, NSH], BF16, tag="pt",
                                             name="pt")
                                nc.scalar.activation(pt[:], e[:], AF.Exp)
                                for qb in range(NIB):
                                    lhs = pt[:, qb * P:(qb + 1) * P]
                                    nc.tensor.matmul(
                                        psa[qb][:], lhs, whs[:, jt, h, 0:N1],
                                        start=(jt == 0), stop=(jt == CH - 1))
                                    nc.tensor.matmul(
                                        psb[qb][:], lhs,
                                        whs[:, jt, h, N1:N1 + 257],
                                        start=(jt == 0), stop=(jt == CH - 1))
                            for qb in range(NIB):
                                if jc == 0:
                                    nc.vector.tensor_copy(
                                        acc[h][:, qb, 0:N1], psa[qb][:])
                                    nc.vector.tensor_copy(
                                        acc[h][:, qb, N1:O + 1], psb[qb][:])
                                else:
                                    nc.vector.scalar_tensor_tensor(
                                        acc[h][:, qb, 0:N1], psa[qb][:], 0.0,
                                        acc[h][:, qb, 0:N1], alu.add, alu.add)
                                    nc.vector.scalar_tensor_tensor(
                                        acc[h][:, qb, N1:O + 1], psb[qb][:],
                                        0.0, acc[h][:, qb, N1:O + 1],
                                        alu.add, alu.add)
                    for h in range(H):
                        for qb in range(NIB):
                            den = sm.tile([P, 1], F32, tag="den", name="den")
                            if mean_heads:
                                nc.vector.tensor_single_scalar(
                                    den[:], acc[h][:, qb, O:O + 1], float(H),
                                    alu.mult)
                            else:
                                nc.vector.tensor_copy(
                                    den[:], acc[h][:, qb, O:O + 1])
                            rcp = sm.tile([P, 1], F32, tag="rcp", name="rcp")
                            nc.vector.reciprocal(rcp[:], den[:])
                            out_ap = (dest[:, qb, 0:O] if mean_heads else
                                      dest[:, qb, h * O:(h + 1) * O])
                            nc.vector.scalar_tensor_tensor(
                                out_ap, acc[h][:, qb, 0:O], rcp[:, 0:1],
                                out_ap, alu.mult, alu.add)

            # ---- poolX: h1pre / h1 / h1T ----
            with tc.tile_pool(name="poolX", bufs=1) as px:
                h1pre = px.tile([P, NIB, F1], F32)

                # ===== Phase A =====
                with (
                    tc.tile_pool(name="phA", bufs=1) as pa,
                    tc.tile_pool(name="phA_ps", bufs=2, space="PSUM") as paps,
                ):
                    a0b = bcast(pa, a0[:, :], H * 2 * HID, "a0")
                    a0b = a0b.rearrange("p (h c) -> p h c", h=H)
                    rp0bb = bcast(pa, rp0b[:, :], F1, "rp0b")
                    nfTsb = pa.tile([P, DIN // P, NSH], F32)
                    nc.sync.dma_start(
                        out=nfTsb[:],
                        in_=nfT.rearrange("(k p) i -> p k i", p=P))
                    nfTbf = pa.tile([P, DIN // P, NSH], BF16)
                    nc.vector.tensor_copy(nfTbf[:], nfTsb[:])
                    s_sb0 = pa.tile([P, H, NIB, 2], F32)

                    for h in range(H):
                        psv = [paps.tile([P, HID], F32, tag=f"wh0ps{ib}",
                                         bufs=1, name=f"wh0ps_{ib}")
                               for ib in range(NIB)]
                        for k in range(DIN // P):
                            w0t = sb.tile([P, HID], F32, tag="w0t",
                                          bufs=3, name="w0t")
                            nc.sync.dma_start(
                                out=w0t[:], in_=W0[h, k * P:(k + 1) * P, :])
                            for ib in range(NIB):
                                nc.tensor.matmul(
                                    psv[ib][:],
                                    nfTsb[:, k, ib * P:(ib + 1) * P],
                                    w0t[:],
                                    start=(k == 0), stop=(k == DIN // P - 1))
                        for ib in range(NIB):
                            ps = psv[ib]
                            whtmp = sb.tile([P, HID], F32, tag="whtmp",
                                            bufs=1, name="whtmp")
                            nc.scalar.copy(whtmp[:], ps[:])
                            for which in range(2):
                                tmp = sb.tile([P, HID], F32, tag="sred",
                                              bufs=1, name="sred")
                                nc.vector.tensor_tensor(
                                    tmp[:], whtmp[:],
                                    a0b[:, h, which * HID:(which + 1) * HID],
                                    alu.mult)
                                nc.vector.tensor_reduce(
                                    s_sb0[:, h, ib, which:which + 1], tmp[:],
                                    mybir.AxisListType.X, alu.add)
                            pack = sb.tile([P, C0], BF16, tag="pack0",
                                           name="pack")
                            nc.vector.tensor_copy(pack[:, 0:HID], whtmp[:])
                            nc.vector.memset(pack[:, HID:HID + 1], 1.0)
                            nc.vector.memset(pack[:, HID + 1:C0], 0.0)
                            nc.sync.dma_start(
                                out=g0_in[h, ib * P:(ib + 1) * P, :],
                                in_=pack[:])
                    nc.sync.dma_start(
                        out=g0s_in.rearrange("h (ib p) c -> p h ib c", p=P),
                        in_=s_sb0[:])
                    nc.gpsimd.collective_compute(
                        "AllGather", alu.bypass, replica_groups=groups,
                        ins=[g0_in[:, :, :].opt()],
                        outs=[g0_out[:, :, :, :].opt()])
                    nc.gpsimd.collective_compute(
                        "AllGather", alu.bypass, replica_groups=groups,
                        ins=[g0s_in[:, :, :].opt()],
                        outs=[g0s_out[:, :, :, :].opt()])

                    rp0wsb = pa.tile([P, DIN // P, F1], BF16)
                    nc.sync.dma_start(
                        out=rp0wsb[:],
                        in_=rp0w.rearrange("(k p) o -> p k o", p=P))
                    for ib in range(NIB):
                        for oc in range(4):
                            ps2 = paps.tile([P, 512], F32, tag="rp0ps",
                                            name="ps2")
                            for k in range(DIN // P):
                                nc.tensor.matmul(
                                    ps2[:], nfTbf[:, k, ib * P:(ib + 1) * P],
                                    rp0wsb[:, k, oc * 512:(oc + 1) * 512],
                                    start=(k == 0), stop=(k == DIN // P - 1))
                            nc.vector.tensor_tensor(
                                h1pre[:, ib, oc * 512:(oc + 1) * 512],
                                ps2[:], rp0bb[:, oc * 512:(oc + 1) * 512],
                                alu.add)

                attention(0, HID, 256, g0_out, g0s_out, g0s_in, h1pre, False)

                h1T = px.tile([P, F1 // P, NSH], F32)
                # ===== LN0 + ELU -> h1, transpose -> h1T =====
                with tc.tile_pool(name="ln0p", bufs=1) as lp0:
                    ln0gb = bcast(lp0, ln0g[:, :], F1, "ln0g")
                    ln0bb = bcast(lp0, ln0b[:, :], F1, "ln0b")
                    for ib in range(NIB):
                        ln_elu(lp0, h1pre[:, ib, :], ln0gb[:, :],
                               ln0bb[:, :], F1, h1pre[:, ib, :], True)
                with tc.tile_pool(name="trps", bufs=2, space="PSUM") as tps:
                    for ib in range(NIB):
                        for fb in range(F1 // P):
                            pst = tps.tile([P, P], F32, tag="pst",
                                           name="pst")
                            nc.tensor.transpose(
                                pst[:], h1pre[:, ib, fb * P:(fb + 1) * P],
                                ident[:])
                            nc.scalar.copy(
                                h1T[:, fb, ib * P:(ib + 1) * P], pst[:])

                # ===== Phase B =====
                with (
                    tc.tile_pool(name="phB", bufs=1) as pb,
                    tc.tile_pool(name="phB_d", bufs=3) as pbd,
                    tc.tile_pool(name="phB_ps", bufs=1, space="PSUM") as pbps,
                ):
                    a1bs = [bcast(pb, a1[:, hh * 2 * DOUT:(hh + 1) * 2 * DOUT],
                                  2 * DOUT, f"a1_{hh}") for hh in range(H)]
                    rp1bb = bcast(pb, rp1b[:, :], DOUT, "rp1b")
                    s_sb1 = pb.tile([P, H, NIB, 2], F32)
                    halves = ((0, 512), (512, DOUT))
                    for h in range(H):
                        psw = [pbps.tile([P, DOUT], F32, tag=f"wh1ps{ib}",
                                         name=f"wh1ps_{ib}")
                               for ib in range(NIB)]
                        for k in range(F1 // P):
                            w1t = pbd.tile([P, DOUT], F32, tag="w1t",
                                           name="w1t")
                            nc.sync.dma_start(
                                out=w1t[:], in_=W1[h, k * P:(k + 1) * P, :])
                            for ib in range(NIB):
                                for (o0, o1) in halves:
                                    nc.tensor.matmul(
                                        psw[ib][:, o0:o1],
                                        h1T[:, k, ib * P:(ib + 1) * P],
                                        w1t[:, o0:o1],
                                        start=(k == 0),
                                        stop=(k == F1 // P - 1))
                        for ib in range(NIB):
                            whtmp1 = sb.tile([P, DOUT], F32, tag="whtmp1",
                                             bufs=1, name="whtmp1")
                            nc.scalar.copy(whtmp1[:], psw[ib][:])
                            for which in range(2):
                                tmp = sb.tile([P, DOUT], F32, tag="sred1",
                                              bufs=1, name="tmp")
                                nc.vector.tensor_tensor(
                                    tmp[:], whtmp1[:],
                                    a1bs[h][:, which * DOUT:(which + 1) * DOUT],
                                    alu.mult)
                                nc.vector.tensor_reduce(
                                    s_sb1[:, h, ib, which:which + 1], tmp[:],
                                    mybir.AxisListType.X, alu.add)
                            pack1 = sb.tile([P, C1], BF16, tag="pack1",
                                            name="pack1")
                            nc.vector.tensor_copy(pack1[:, 0:DOUT],
                                                  whtmp1[:])
                            nc.vector.memset(pack1[:, DOUT:DOUT + 1], 1.0)
                            nc.vector.memset(pack1[:, DOUT + 1:C1], 0.0)
                            nc.sync.dma_start(
                                out=g1_in[h, ib * P:(ib + 1) * P, :],
                                in_=pack1[:])
                    nc.sync.dma_start(
                        out=g1s_in.rearrange("h (ib p) c -> p h ib c", p=P),
                        in_=s_sb1[:])
                    nc.gpsimd.collective_compute(
                        "AllGather", alu.bypass, replica_groups=groups,
                        ins=[g1_in[:, :, :].opt()],
                        outs=[g1_out[:, :, :, :].opt()])
                    nc.gpsimd.collective_compute(
                        "AllGather", alu.bypass, replica_groups=groups,
                        ins=[g1s_in[:, :, :].opt()],
                        outs=[g1s_out[:, :, :, :].opt()])

                    psr = [pbps.tile([P, DOUT], F32, tag=f"wh1ps{ib}",
                                     name=f"rp1ps_{ib}")
                           for ib in range(NIB)]
                    for k in range(F1 // P):
                        r1t = pbd.tile([P, DOUT], F32, tag="r1t",
                                       name="r1t")
                        nc.sync.dma_start(
                            out=r1t[:], in_=rp1w[k * P:(k + 1) * P, :])
                        for ib in range(NIB):
                            for (o0, o1) in halves:
                                nc.tensor.matmul(
                                    psr[ib][:, o0:o1],
                                    h1T[:, k, ib * P:(ib + 1) * P],
                                    r1t[:, o0:o1],
                                    start=(k == 0), stop=(k == F1 // P - 1))
                    for ib in range(NIB):
                        nc.vector.tensor_tensor(
                            h2pre[:, ib, :], psr[ib][:], rp1bb[:, :],
                            alu.add)

            attention(1, DOUT, 512, g1_out, g1s_out, g1s_in, h2pre, True)

            # ===== LN1 -> h2 out =====
            with tc.tile_pool(name="ln1p", bufs=1) as lp1:
                ln1gb = bcast(lp1, ln1g[:, :], DOUT, "ln1g")
                ln1bb = bcast(lp1, ln1b[:, :], DOUT, "ln1b")
                for ib in range(NIB):
                    o = sb.tile([P, DOUT], F32, tag="hout", name="o")
                    ln_elu(lp1, h2pre[:, ib, :], ln1gb[:, :], ln1bb[:, :],
                           DOUT, o[:], False)
                    nc.sync.dma_start(out=h2[ib * P:(ib + 1) * P, :],
                                      in_=o[:])

    nc.finalize()
    return nc


_NC_CACHE = None


def _get_nc():
    global _NC_CACHE
    if _NC_CACHE is None:
        _NC_CACHE = build_nc()
    return _NC_CACHE


def build_in_maps(node_features, adjacency, edge_weights, W0, a0, W1, a1,
                  rp0_w, rp0_b, rp1_w, rp1_b, ln0_g, ln0_b, ln1_g, ln1_b):
    bf = ml_dtypes.bfloat16
    node_features = np.asarray(node_features, np.float32)
    adjacency = np.asarray(adjacency)
    edge_weights = np.asarray(edge_weights, np.float32)

    nfT = np.ascontiguousarray(node_features.T)
    w0 = np.ascontiguousarray(np.asarray(W0, np.float32))
    w1 = np.ascontiguousarray(np.asarray(W1, np.float32))
    rp0w = np.asarray(rp0_w, np.float32).astype(bf)
    rp1w = np.ascontiguousarray(np.asarray(rp1_w, np.float32))

    def row(x, w):
        return np.ascontiguousarray(np.asarray(x, np.float32)).reshape(1, w)

    in_maps = []
    for c in range(NCORES):
        rows = slice(c * NSH, (c + 1) * NSH)
        adj = np.asarray(adjacency[rows, :])
        conn = adj != 0
        conn[np.arange(NSH), c * NSH + np.arange(NSH)] = True
        ew = edge_weights[rows, :]
        ewp = np.where(conn, ew, np.float32(0.0)).astype(np.float32)
        ewn = np.where(conn, np.float32(0.0),
                       np.float32(NEGBIG) * ew).astype(bf)
        in_maps.append({
            "nfT": np.ascontiguousarray(nfT[:, rows]),
            "ewpT": np.ascontiguousarray(ewp.T),
            "ewnT": np.ascontiguousarray(ewn.T),
            "W0": w0, "a0": np.asarray(a0, np.float32).reshape(1, -1),
            "W1": w1, "a1": np.asarray(a1, np.float32).reshape(1, -1),
            "rp0w": rp0w, "rp0b": row(rp0_b, F1),
            "rp1w": rp1w, "rp1b": row(rp1_b, DOUT),
            "ln0g": row(ln0_g, F1), "ln0b": row(ln0_b, F1),
            "ln1g": row(ln1_g, DOUT), "ln1b": row(ln1_b, DOUT),
        })
    return in_maps


def kernel(**inputs):
    in_maps = build_in_maps(**inputs)
    nc = _get_nc()
    res = run_bass_kernel_spmd(nc, in_maps, list(range(NCORES)))
    return np.concatenate([res.results[c]["h2"] for c in range(NCORES)],
                          axis=0).astype(np.float32)

